# revision 1
# baseline (speedup 1.0000x reference)
"""BondInfluenceSelfAttention TRN2 kernel.

Full-input contract: kernel(**inputs) takes the complete unsharded inputs and
returns the full [B, L, D] output. Internally shards across 8 NeuronCores:
core c handles batch b = c // 4 and head-group g = c % 4 (4 heads, 256 dk dims).
Each core computes its heads' attention plus the partial output projection
through its 256 rows of Wo; the host sums the 4 partials per batch and adds bo.

Device-side formulation (per core), all matmuls in fp32r:
  QT = (Wq_g/8)^T x^T   [256, L]   (1/sqrt(dk)=1/8 folded into Wq/bq on host)
  KT = Wk_g^T x^T       [256, L]
  V  = x Wv_g           [L, 256]   (bias via an appended ones-row matmul)
  S^T tile = K Q^T      [L_k, L_q] (per head, dk=64 contraction)
  P~ = exp(S^T * bondT) (no max subtraction: |scores| <~ 3, fp32 exp is safe)
  Head pairs share one 2-bank PSUM scores tile so the bond multiply and exp
  run as single [128, 1024] DVE/ACT ops (bond broadcast via a step-0 free-dim
  AP). O^T accumulates per head with a ones column appended to V, putting the
  softmax denominator in row 64 of each accumulator. Reciprocals stay on
  lane 64; K=1 broadcast matmuls expand them for the normalize multiply; the
  output projection is interleaved per chunk (yp tiles reuse freed oacc
  PSUM slots). Finally Y = O Wo_g, summed with the other cores on the host.
"""

import numpy as np

try:
    import concourse.bass as bass  # noqa: F401
except ImportError:  # pragma: no cover
    import sys

    sys.path.insert(0, "/opt/trn_rl_repo")

import concourse.bacc as bacc
import concourse.mybir as mybir
import concourse.tile as tile
from concourse.bass_utils import run_bass_kernel_spmd

F32 = mybir.dt.float32
F32R = mybir.dt.float32r
F16 = mybir.dt.float16

D = 1024  # d_model
L = 2048  # sequence length
B = 2  # batch
HPC = 4  # heads per core
DKG = 256  # dk dims per core (4 heads x 64)
NK = D // 128  # 8 contraction k-tiles for the projections
LT = L // 128  # 16 L-tiles
NCH = L // 512  # 4 L_q chunks
N_CORES = 8

_CACHED_NC = None


def _build_nc():
    nc = bacc.Bacc("TRN2", target_bir_lowering=False, debug=False, num_devices=N_CORES)

    xt_d = nc.declare_dram_parameter("xt", [D, L], F32R, isOutput=False)
    bd_d = nc.declare_dram_parameter("bd", [L, L], F16, isOutput=False)
    wq_d = nc.declare_dram_parameter("wq", [D, DKG], F32R, isOutput=False)
    wk_d = nc.declare_dram_parameter("wk", [D, DKG], F32R, isOutput=False)
    wv_d = nc.declare_dram_parameter("wv", [D, DKG], F32R, isOutput=False)
    bqk_d = nc.declare_dram_parameter("bqk", [128, 4], F32, isOutput=False)
    bv_d = nc.declare_dram_parameter("bv", [1, DKG], F32R, isOutput=False)
    wo_d = nc.declare_dram_parameter("wo", [DKG, D], F32R, isOutput=False)
    y_d = nc.declare_dram_parameter("y", [L, D], F32, isOutput=True)

    Exp = mybir.ActivationFunctionType.Exp
    Identity = mybir.ActivationFunctionType.Identity

    with tile.TileContext(nc) as tc:
        with tc.tile_pool(name="persist", bufs=1) as pp:
            qt = [pp.tile([128, L], F32R, tag=f"qt{t}", name=f"qt{t}") for t in range(2)]
            kt = [pp.tile([128, L], F32R, tag=f"kt{t}", name=f"kt{t}") for t in range(2)]
            vt = [
                pp.tile([128, HPC, 65], F32R, tag=f"v{i}", name=f"v{i}")
                for i in range(LT)
            ]
            ot = [pp.tile([128, L], F32R, tag=f"ot{t}", name=f"ot{t}") for t in range(2)]
            wo_sb = pp.tile([128, 2, D], F32R, tag="wo", name="wo_sb")
            bqk_sb = pp.tile([128, 4], F32, tag="bqk", name="bqk_sb")
            bv_sb = pp.tile([1, DKG], F32R, tag="bv", name="bv_sb")
            onesv_f = pp.tile([1, 128], F32, tag="onesvf", name="onesv_f")
            onesv = pp.tile([1, 128], F32R, tag="onesv", name="onesv")
            onesb_f = pp.tile([128, 128], F32, tag="onesbf", name="onesb_f")
            onesb = pp.tile([128, 128], F32R, tag="onesb", name="onesb")

            nc.sync.dma_start(out=wo_sb, in_=wo_d.ap().rearrange("(t p) n -> p t n", p=128))
            nc.sync.dma_start(out=bqk_sb, in_=bqk_d[:, :])
            nc.sync.dma_start(out=bv_sb, in_=bv_d[:, :])
            nc.vector.memset(onesv_f, 1.0)
            nc.vector.tensor_copy(out=onesv, in_=onesv_f)
            nc.vector.memset(onesb_f, 1.0)
            nc.vector.tensor_copy(out=onesb, in_=onesb_f)

            # ---------------- Phase 1: projections ----------------
            with tc.tile_pool(name="xw", bufs=1) as xw, tc.tile_pool(
                name="ps1", bufs=2, space="PSUM"
            ) as ps1:
                xk = [
                    xw.tile([128, L], F32R, tag=f"x{k}", name=f"x{k}") for k in range(NK)
                ]
                xt_t = xt_d.ap().rearrange("(k p) l -> k p l", p=128)
                dma_engines = [nc.sync, nc.scalar]
                for k in range(NK):
                    dma_engines[k % 2].dma_start(out=xk[k], in_=xt_t[k])
                wq_sb = xw.tile([128, NK, DKG], F32R, tag="wq", name="wq_sb")
                wk_sb = xw.tile([128, NK, DKG], F32R, tag="wk", name="wk_sb")
                wv_sb = xw.tile([128, NK, DKG], F32R, tag="wv", name="wv_sb")
                nc.sync.dma_start(out=wq_sb, in_=wq_d.ap().rearrange("(k p) n -> p k n", p=128))
                nc.sync.dma_start(out=wk_sb, in_=wk_d.ap().rearrange("(k p) n -> p k n", p=128))
                nc.sync.dma_start(out=wv_sb, in_=wv_d.ap().rearrange("(k p) n -> p k n", p=128))

                for t in range(2):
                    for c in range(NCH):
                        pq = ps1.tile([128, 512], F32, tag="pq", name="pq")
                        for k in range(NK):
                            nc.tensor.matmul(
                                pq[:, :],
                                wq_sb[:, k, 128 * t : 128 * (t + 1)],
                                xk[k][:, 512 * c : 512 * (c + 1)],
                                start=(k == 0),
                                stop=(k == NK - 1),
                            )
                        nc.scalar.activation(
                            out=qt[t][:, 512 * c : 512 * (c + 1)],
                            in_=pq[:, :],
                            func=Identity,
                            bias=bqk_sb[:, t : t + 1],
                        )
                        pk = ps1.tile([128, 512], F32, tag="pk", name="pk")
                        for k in range(NK):
                            nc.tensor.matmul(
                                pk[:, :],
                                wk_sb[:, k, 128 * t : 128 * (t + 1)],
                                xk[k][:, 512 * c : 512 * (c + 1)],
                                start=(k == 0),
                                stop=(k == NK - 1),
                            )
                        nc.scalar.activation(
                            out=kt[t][:, 512 * c : 512 * (c + 1)],
                            in_=pk[:, :],
                            func=Identity,
                            bias=bqk_sb[:, 2 + t : 3 + t],
                        )

                for i in range(LT):
                    pv = ps1.tile([128, DKG], F32, tag="pv", name="pv")
                    for k in range(NK):
                        nc.tensor.matmul(
                            pv[:, :],
                            xk[k][:, 128 * i : 128 * (i + 1)],
                            wv_sb[:, k, :],
                            start=(k == 0),
                            stop=False,
                        )
                    nc.tensor.matmul(
                        pv[:, :], onesv[:, :], bv_sb[:, :], start=False, stop=True
                    )
                    nc.scalar.activation(
                        out=vt[i][:, :, 0:64],
                        in_=pv.rearrange("p (h e) -> p h e", e=64),
                        func=Identity,
                    )
                    nc.vector.memset(vt[i][:, :, 64:65].bitcast(F32), 1.0)
                    nc.vector.tensor_copy(
                        out=vt[i][:, :, 64:65], in_=vt[i][:, :, 64:65].bitcast(F32)
                    )

            # ------- Phase 2+3: attention with interleaved output projection -------
            with tc.tile_pool(name="att", bufs=1) as att, tc.tile_pool(
                name="ps2", bufs=1, space="PSUM"
            ) as ps2:
                for c in range(NCH):
                    oaccs = [
                        ps2.tile([65, 512], F32, tag="oacc", bufs=4, name=f"oacc{h}")
                        for h in range(HPC)
                    ]
                    for i in range(LT):
                        bt = att.tile([128, 512], F16, tag="bond", bufs=4, name="bt")
                        nc.sync.dma_start(
                            out=bt,
                            in_=bd_d[128 * i : 128 * (i + 1), 512 * c : 512 * (c + 1)],
                        )
                        bt_bcast = bass.AP(
                            tensor=bt.tensor,
                            offset=bt.offset,
                            ap=[bt.ap[0], [0, 2]] + list(bt.ap[1:]),
                        )
                        ptp = []
                        for t in range(2):
                            spair = ps2.tile(
                                [128, 2, 512], F32, tag="s", bufs=2, name="spair"
                            )
                            for half in range(2):
                                nc.tensor.matmul(
                                    spair[:, half, :],
                                    kt[t][64 * half : 64 * (half + 1), 128 * i : 128 * (i + 1)],
                                    qt[t][64 * half : 64 * (half + 1), 512 * c : 512 * (c + 1)],
                                    start=True,
                                    stop=True,
                                )
                            sbpair = att.tile([128, 2, 512], F32, tag="sb", bufs=3, name="sbpair")
                            nc.vector.tensor_mul(out=sbpair, in0=spair, in1=bt_bcast)
                            pt_t = att.tile([128, 2, 512], F32R, tag="pt", bufs=3, name="pt_t")
                            nc.scalar.activation(out=pt_t, in_=sbpair, func=Exp)
                            ptp.append(pt_t)
                        for h in range(HPC):
                            t, half = h // 2, h % 2
                            nc.tensor.matmul(
                                oaccs[h][:, :],
                                vt[i][:, h, :],
                                ptp[t][:, half, :],
                                start=(i == 0),
                                stop=(i == LT - 1),
                            )
                    # chunk tail: denominators sit on lane 64 of each oacc
                    rd = att.tile([65, HPC, 512], F32R, tag="rd", bufs=2, name="rd")
                    with nc.allow_low_precision(reason="f32r is full fp32 storage"):
                        for h in range(HPC):
                            nc.vector.reciprocal(
                                out=rd[64:65, h, :],
                                in_=oaccs[h][64:65, :],
                            )
                    for t in range(2):
                        for half in range(2):
                            h = 2 * t + half
                            bc = ps2.tile([64, 512], F32, tag="s", bufs=2, name="bc")
                            nc.tensor.matmul(
                                bc[:, :],
                                onesb[64:65, 0:64],
                                rd[64:65, h, :],
                                start=True,
                                stop=True,
                                tile_position=(64, 0),
                            )
                            bcs = att.tile([64, 512], F32, tag="bcs", bufs=3, name="bcs")
                            nc.scalar.activation(out=bcs, in_=bc[:, :], func=Identity)
                            if half == 0:
                                nc.vector.tensor_mul(
                                    out=ot[t][0:64, 512 * c : 512 * (c + 1)],
                                    in0=oaccs[h][0:64, :],
                                    in1=bcs,
                                )
                            else:
                                odd = att.tile([64, 512], F32R, tag="odd", bufs=2, name="odd")
                                nc.vector.tensor_mul(
                                    out=odd,
                                    in0=oaccs[h][0:64, :],
                                    in1=bcs,
                                )
                                nc.sync.dma_start(
                                    out=ot[t][64:128, 512 * c : 512 * (c + 1)],
                                    in_=odd,
                                )
                    # output projection for this chunk's four L-tiles
                    for j in range(4 * c, 4 * c + 4):
                        for dh in range(2):
                            yp = ps2.tile([128, 512], F32, tag="oacc", bufs=4, name="yp")
                            for t in range(2):
                                nc.tensor.matmul(
                                    yp[:, :],
                                    ot[t][:, 128 * j : 128 * (j + 1)],
                                    wo_sb[:, t, 512 * dh : 512 * (dh + 1)],
                                    start=(t == 0),
                                    stop=(t == 1),
                                )
                            ys = att.tile([128, 512], F32, tag="ys", bufs=4, name="ys")
                            nc.scalar.activation(out=ys, in_=yp[:, :], func=Identity)
                            nc.sync.dma_start(
                                out=y_d[128 * j : 128 * (j + 1), 512 * dh : 512 * (dh + 1)],
                                in_=ys,
                            )

    nc.compile()
    return nc


def _get_nc():
    global _CACHED_NC
    if _CACHED_NC is None:
        _CACHED_NC = _build_nc()
    return _CACHED_NC


def _host_prep(x, bond_influence, Wq, bq, Wk, bk, Wv, bv, Wo):
    in_maps = []
    for core in range(N_CORES):
        b, g = core // HPC, core % HPC
        s = slice(g * DKG, (g + 1) * DKG)
        bq_g = (bq[s] / 8.0).astype(np.float32)
        bk_g = bk[s].astype(np.float32)
        bqk = np.stack(
            [bq_g[0:128], bq_g[128:256], bk_g[0:128], bk_g[128:256]], axis=1
        )
        in_maps.append(
            {
                "xt": np.ascontiguousarray(x[b].T),
                "bd": np.ascontiguousarray(bond_influence[b].T.astype(np.float16)),
                "wq": np.ascontiguousarray(Wq[:, s] / 8.0),
                "wk": np.ascontiguousarray(Wk[:, s]),
                "wv": np.ascontiguousarray(Wv[:, s]),
                "bqk": np.ascontiguousarray(bqk),
                "bv": np.ascontiguousarray(bv[s][None, :]),
                "wo": np.ascontiguousarray(Wo[s, :]),
            }
        )
    return in_maps


def kernel(
    x,
    bond_influence,
    Wq,
    bq,
    Wk,
    bk,
    Wv,
    bv,
    Wo,
    bo,
    _trace=False,
    _trace_out=None,
):
    x = np.asarray(x, dtype=np.float32)
    bond_influence = np.asarray(bond_influence, dtype=np.float32)
    args = [np.asarray(a, dtype=np.float32) for a in (Wq, bq, Wk, bk, Wv, bv, Wo)]
    bo = np.asarray(bo, dtype=np.float32)

    nc = _get_nc()
    in_maps = _host_prep(x, bond_influence, *args)
    kwargs = {}
    if _trace:
        kwargs = dict(trace=True, tmpdir=_trace_out)
    res = run_bass_kernel_spmd(nc, in_maps, list(range(N_CORES)), **kwargs)

    out = np.zeros((B, L, D), dtype=np.float32)
    for b in range(B):
        acc = res.results[4 * b]["y"].astype(np.float32).copy()
        for g in range(1, HPC):
            acc += res.results[4 * b + g]["y"]
        out[b] = acc + bo[None, :]
    if _trace:
        return out, res
    return out



# revision 10
# speedup vs baseline: 1.0826x; 1.0826x over previous
"""BondInfluenceSelfAttention TRN2 kernel (v2).

Full-input contract: kernel(**inputs) takes the complete unsharded inputs and
returns the full [B, L, D] output. Internally shards across 8 NeuronCores:
core c handles batch b = c // 4 and head-group g = c % 4 (4 heads, 256 dk
dims). Each core computes its heads' attention plus the partial output
projection through its 256 rows of Wo; the host sums the 4 partials per batch
and adds bo.

v2 design (vs the v1 in git history):
- bf16 operands for every matmul (proj, scores, PV, out-proj); fp32 PSUM.
- Score matmuls (K=64) packed two-per-array via tile_position row groups;
  PV matmuls (M=64) packed two-per-array via column groups.
- Softmax denominators from 4-way column-packed ones^T @ P matmuls into one
  PSUM bank; normalization deferred to the chunk tail (reciprocal_approx_fast
  on DVE + K=1 broadcast matmuls), killing v1's 53us of DVE RECIPROCAL.
- ACT does only: exp in [128,4096] ops, Q/K/V PSUM->SBUF copies (with bias),
  Y copies. DVE does only: bond-multiply (PSUM->SBUF), recip, normalize.
- QKV projection matmul groups are interleaved into chunk 0's attention steps
  (just-in-time spans) so DVE/ACT never sit idle behind a serial proj phase.
"""

import numpy as np

try:
    import concourse.bass as bass  # noqa: F401
except ImportError:  # pragma: no cover
    import sys

    sys.path.insert(0, "/opt/trn_rl_repo")
    import concourse.bass as bass  # noqa: F401

import concourse.bacc as bacc
import concourse.mybir as mybir
import concourse.tile as tile
from concourse.bass_utils import run_bass_kernel_spmd
from ml_dtypes import bfloat16

F32 = mybir.dt.float32
F32R = mybir.dt.float32r
BF16 = mybir.dt.bfloat16
F16 = mybir.dt.float16

D = 1024  # d_model
L = 2048  # sequence length
B = 2  # batch
HPC = 4  # heads per core
DKG = 256  # dk dims per core (4 heads x 64)
NK = D // 128  # 8 contraction k-tiles for the projections
LT = L // 128  # 16 L k-position tiles
NCH = L // 512  # 4 L_q chunks
NSP = 4  # 512-wide k-position spans (kt/vt production granularity)
N_CORES = 8

_CACHED_NC = None


def _build_nc():
    nc = bacc.Bacc("TRN2", target_bir_lowering=False, debug=False, num_devices=N_CORES)

    xt_d = nc.declare_dram_parameter("xt", [D, L], BF16, isOutput=False)
    bd_d = nc.declare_dram_parameter("bd", [L, L], F16, isOutput=False)
    wq_d = nc.declare_dram_parameter("wq", [D, DKG], BF16, isOutput=False)
    wk_d = nc.declare_dram_parameter("wk", [D, DKG], BF16, isOutput=False)
    wv_d = nc.declare_dram_parameter("wv", [D, DKG], BF16, isOutput=False)
    bqk_d = nc.declare_dram_parameter("bqk", [128, 4], F32, isOutput=False)
    bv_d = nc.declare_dram_parameter("bv", [1, DKG], BF16, isOutput=False)
    wo_d = nc.declare_dram_parameter("wo", [DKG, D], BF16, isOutput=False)
    y_d = nc.declare_dram_parameter("y", [L, D], BF16, isOutput=True)

    Exp = mybir.ActivationFunctionType.Exp
    Identity = mybir.ActivationFunctionType.Identity

    with tile.TileContext(nc) as tc:
        with tc.tile_pool(name="persist", bufs=1) as pp, tc.tile_pool(
            name="work", bufs=1
        ) as wk_pool, tc.tile_pool(name="ps", bufs=1, space="PSUM") as ps:
            # ---- persistent SBUF ----
            xk = [
                pp.tile([128, NK, 512], BF16, tag=f"xk{s}", name=f"xk{s}")
                for s in range(NSP)
            ]
            wq_sb = pp.tile([128, NK, DKG], BF16, tag="wq", name="wq_sb")
            wk_sb = pp.tile([128, NK, DKG], BF16, tag="wk", name="wk_sb")
            wv_sb = pp.tile([128, NK, DKG], BF16, tag="wv", name="wv_sb")
            qt = [pp.tile([128, L], BF16, tag=f"qt{t}", name=f"qt{t}") for t in range(2)]
            kt = [pp.tile([128, L], BF16, tag=f"kt{t}", name=f"kt{t}") for t in range(2)]
            vt = pp.tile([128, LT, DKG], BF16, tag="vt", name="vt")
            st = [
                pp.tile([128, 4, 1024], F32, tag=f"st{g}", name=f"st{g}")
                for g in range(2)
            ]
            pt = [
                pp.tile([128, 4, 1024], BF16, tag=f"pt{g}", name=f"pt{g}")
                for g in range(2)
            ]
            wo_sb = pp.tile([128, 2, D], BF16, tag="wo", name="wo_sb")
            bqk_sb = pp.tile([128, 4], F32, tag="bqk", name="bqk_sb")
            bv_sb = pp.tile([1, DKG], BF16, tag="bv", name="bv_sb")
            ones_r = pp.tile([1, 128], BF16, tag="onesr", name="ones_r")
            onesb = pp.tile([128, 128], BF16, tag="onesb", name="onesb")
            ones_f = pp.tile([128, 128], F32, tag="onesf", name="ones_f")
            # sel[:, t, :]: K=128 selection matrix broadcasting denominator
            # rows {64t, 64t+32} of dsb to output partitions [0:64], [64:128]
            sel = pp.tile([128, 2, 128], F32R, tag="sel", name="sel")
            sel_f = pp.tile([128, 2, 128], F32, tag="self", name="sel_f")

            # ---- input DMA ----
            xt_t = xt_d  # [1024, 2048] : rows = d (k-tiles of 128), cols = L
            for s in range(NSP):
                for k in range(NK):
                    nc.sync.dma_start(
                        out=xk[s][:, k, :],
                        in_=xt_t[128 * k : 128 * (k + 1), 512 * s : 512 * (s + 1)],
                    )
            nc.sync.dma_start(out=wq_sb, in_=wq_d.ap().rearrange("(k p) n -> p k n", p=128))
            nc.sync.dma_start(out=wk_sb, in_=wk_d.ap().rearrange("(k p) n -> p k n", p=128))
            nc.sync.dma_start(out=wv_sb, in_=wv_d.ap().rearrange("(k p) n -> p k n", p=128))
            nc.sync.dma_start(out=wo_sb, in_=wo_d.ap().rearrange("(t p) n -> p t n", p=128))
            nc.sync.dma_start(out=bqk_sb, in_=bqk_d[:, :])
            nc.sync.dma_start(out=bv_sb, in_=bv_d[:, :])
            nc.vector.memset(ones_f, 1.0)
            nc.vector.tensor_copy(out=onesb, in_=ones_f)
            nc.vector.tensor_copy(out=ones_r, in_=ones_f[0:1, :])
            nc.vector.memset(sel_f, 0.0)
            for t in range(2):
                nc.vector.memset(sel_f[64 * t : 64 * t + 1, t, 0:64], 1.0)
                nc.vector.memset(sel_f[64 * t + 32 : 64 * t + 33, t, 64:128], 1.0)
            nc.vector.tensor_copy(out=sel, in_=sel_f)

            # ---- projection group emitters ----
            def kt_group(t, s):
                pb = ps.tile([128, 512], F32, tag="pj", name="pj")
                for k in range(NK):
                    nc.tensor.matmul(
                        pb[:, :],
                        wk_sb[:, k, 128 * t : 128 * (t + 1)],
                        xk[s][:, k, :],
                        start=(k == 0),
                        stop=(k == NK - 1),
                    )
                nc.scalar.activation(
                    out=kt[t][:, 512 * s : 512 * (s + 1)],
                    in_=pb[:, :],
                    func=Identity,
                    bias=bqk_sb[:, 2 + t : 3 + t],
                )

            def qt_group(t, c):
                pb = ps.tile([128, 512], F32, tag="pj", name="pj")
                for k in range(NK):
                    nc.tensor.matmul(
                        pb[:, :],
                        wq_sb[:, k, 128 * t : 128 * (t + 1)],
                        xk[c][:, k, :],
                        start=(k == 0),
                        stop=(k == NK - 1),
                    )
                nc.scalar.activation(
                    out=qt[t][:, 512 * c : 512 * (c + 1)],
                    in_=pb[:, :],
                    func=Identity,
                    bias=bqk_sb[:, t : t + 1],
                )

            def vt_group(ii):  # i-tiles 2*ii, 2*ii+1
                pb = ps.tile([128, 512], F32, tag="pj", name="pj")
                for j in range(2):
                    i = 2 * ii + j
                    s, lo = i // 4, (i % 4) * 128
                    for k in range(NK):
                        nc.tensor.matmul(
                            pb[:, 256 * j : 256 * (j + 1)],
                            xk[s][:, k, lo : lo + 128],
                            wv_sb[:, k, :],
                            start=(k == 0),
                            stop=False,
                        )
                    nc.tensor.matmul(
                        pb[:, 256 * j : 256 * (j + 1)],
                        ones_r,
                        bv_sb,
                        start=False,
                        stop=True,
                    )
                nc.scalar.activation(
                    out=vt[:, 2 * ii : 2 * ii + 2, :],
                    in_=pb.rearrange("p (j n) -> p j n", j=2),
                    func=Identity,
                )

            # just-in-time schedule: chunk-0 steps produce the remaining spans
            slots = {
                (0, 0): [lambda: vt_group(0)],
                (0, 1): [lambda: vt_group(1)],
                (0, 2): [lambda: kt_group(0, 1)],
                (0, 3): [lambda: kt_group(1, 1)],
                (0, 4): [lambda: vt_group(2)],
                (0, 5): [lambda: vt_group(3)],
                (0, 6): [lambda: kt_group(0, 2)],
                (0, 7): [lambda: kt_group(1, 2)],
                (0, 8): [lambda: vt_group(4)],
                (0, 9): [lambda: vt_group(5)],
                (0, 10): [lambda: kt_group(0, 3)],
                (0, 11): [lambda: kt_group(1, 3)],
                (0, 12): [lambda: vt_group(6)],
                (0, 13): [lambda: vt_group(7)],
                (0, 14): [lambda: qt_group(0, 1)],
                (0, 15): [lambda: qt_group(1, 1)],
                (1, 2): [lambda: qt_group(0, 2)],
                (1, 6): [lambda: qt_group(1, 2)],
                (2, 2): [lambda: qt_group(0, 3)],
                (2, 6): [lambda: qt_group(1, 3)],
            }

            # bootstrap: kt span 0 + qt chunk 0 (vt 0..3 go in slots (0,0)/(0,1))
            kt_group(0, 0)
            kt_group(1, 0)
            qt_group(0, 0)
            qt_group(1, 0)

            # ---- bond DMA ring ----
            steps = [(c, i) for c in range(NCH) for i in range(LT)]
            bts = {}

            def bond_dma(n):
                if n >= len(steps):
                    return
                c, i = steps[n]
                bt = wk_pool.tile([128, 512], F16, tag="bt", bufs=6, name="bt")
                nc.sync.dma_start(
                    out=bt,
                    in_=bd_d[128 * i : 128 * (i + 1), 512 * c : 512 * (c + 1)],
                )
                bts[n] = bt

            for n in range(3):
                bond_dma(n)

            # ---- attention ----
            def pv_step(c, j, oacc, dn):
                first, last = (j == 0), (j == LT - 1)
                g = (j // 2) % 2
                for t in range(2):
                    idx = (j % 2) * 2 + t
                    for half in range(2):
                        h = 2 * t + half
                        nc.tensor.matmul(
                            oacc[t][64 * half : 64 * (half + 1), :],
                            vt[:, j, 64 * h : 64 * (h + 1)],
                            pt[g][:, idx, 512 * half : 512 * (half + 1)],
                            start=first,
                            stop=last,
                            tile_position=(0, 64 * half),
                        )
                for h in range(HPC):
                    t, half = h // 2, h % 2
                    idx = (j % 2) * 2 + t
                    nc.tensor.matmul(
                        dn[32 * h : 32 * h + 1, :],
                        onesb[:, 0:1],
                        pt[g][:, idx, 512 * half : 512 * (half + 1)],
                        start=first,
                        stop=last,
                        tile_position=(0, 32 * h),
                    )

            for c in range(NCH):
                oacc = [
                    ps.tile([128, 512], F32, tag=f"o{t}", name=f"oacc{t}")
                    for t in range(2)
                ]
                dn = ps.tile([128, 512], F32, tag="dn", name="dn")
                for i in range(LT):
                    n = c * LT + i
                    for fn in slots.get((c, i), ()):
                        fn()
                    bond_dma(n + 3)
                    bt = bts.pop(n)
                    bt_b = bass.AP(
                        tensor=bt.tensor,
                        offset=bt.offset,
                        ap=[bt.ap[0], [0, 2]] + list(bt.ap[1:]),
                    )
                    g = (i // 2) % 2
                    for t in range(2):
                        sp = ps.tile([128, 2, 512], F32, tag="s", bufs=2, name="sp")
                        nc.tensor.matmul(
                            sp[:, 0, :],
                            kt[t][0:64, 128 * i : 128 * (i + 1)],
                            qt[t][0:64, 512 * c : 512 * (c + 1)],
                            start=True,
                            stop=True,
                        )
                        nc.tensor.matmul(
                            sp[:, 1, :],
                            kt[t][64:128, 128 * i : 128 * (i + 1)],
                            qt[t][64:128, 512 * c : 512 * (c + 1)],
                            start=True,
                            stop=True,
                            tile_position=(64, 0),
                        )
                        idx = (i % 2) * 2 + t
                        out_view = st[g][:, idx, :].rearrange("p (h q) -> p h q", h=2)
                        nc.vector.tensor_mul(out=out_view, in0=sp, in1=bt_b)
                    if i % 2 == 1:
                        with nc.allow_low_precision(reason="bf16 probs"):
                            nc.scalar.activation(out=pt[g], in_=st[g], func=Exp)
                    if i >= 2:
                        pv_step(c, i - 2, oacc, dn)
                pv_step(c, LT - 2, oacc, dn)
                pv_step(c, LT - 1, oacc, dn)

                # ---- chunk tail: denominators, normalize, out-proj ----
                dsb = wk_pool.tile([128, 512], F32R, tag="dsb", bufs=2, name="dsb")
                nc.vector.tensor_copy(out=dsb, in_=dn)
                bcb = [
                    ps.tile([128, 512], F32, tag=("dn" if t == 0 else "pj"), name="bcb")
                    for t in range(2)
                ]
                for t in range(2):
                    nc.tensor.matmul(
                        bcb[t][:, :],
                        sel[:, t, :],
                        dsb[:, :],
                        start=True,
                        stop=True,
                    )
                rb = [
                    wk_pool.tile([128, 512], F32, tag="rb", bufs=2, name="rb")
                    for _ in range(2)
                ]
                on = wk_pool.tile([128, 2, 512], BF16, tag="on", bufs=2, name="on")
                with nc.allow_low_precision(reason="bf16 normalized O"):
                    for t in range(2):
                        nc.vector.reciprocal_approx_fast(out=rb[t], in_=bcb[t])
                        nc.vector.tensor_mul(out=on[:, t, :], in0=oacc[t], in1=rb[t])
                for jl in range(4):
                    j = 4 * c + jl
                    for dh in range(2):
                        yp = ps.tile([128, 512], F32, tag="pj", name="yp")
                        for t in range(2):
                            nc.tensor.matmul(
                                yp[:, :],
                                on[:, t, 128 * jl : 128 * (jl + 1)],
                                wo_sb[:, t, 512 * dh : 512 * (dh + 1)],
                                start=(t == 0),
                                stop=(t == 1),
                            )
                        ys = wk_pool.tile([128, 512], BF16, tag="ys", bufs=4, name="ys")
                        with nc.allow_low_precision(reason="bf16 partial Y"):
                            nc.scalar.activation(out=ys, in_=yp, func=Identity)
                        nc.gpsimd.dma_start(
                            out=y_d[128 * j : 128 * (j + 1), 512 * dh : 512 * (dh + 1)],
                            in_=ys,
                        )

    nc.compile()
    return nc


def _get_nc():
    global _CACHED_NC
    if _CACHED_NC is None:
        _CACHED_NC = _build_nc()
    return _CACHED_NC


def _host_prep(x, bond_influence, Wq, bq, Wk, bk, Wv, bv, Wo):
    in_maps = []
    for core in range(N_CORES):
        b, g = core // HPC, core % HPC
        s = slice(g * DKG, (g + 1) * DKG)
        bq_g = (bq[s] / 8.0).astype(np.float32)
        bk_g = bk[s].astype(np.float32)
        bqk = np.stack(
            [bq_g[0:128], bq_g[128:256], bk_g[0:128], bk_g[128:256]], axis=1
        )
        in_maps.append(
            {
                "xt": np.ascontiguousarray(x[b].T).astype(bfloat16),
                "bd": np.ascontiguousarray(bond_influence[b].T.astype(np.float16)),
                "wq": np.ascontiguousarray(Wq[:, s] / 8.0).astype(bfloat16),
                "wk": np.ascontiguousarray(Wk[:, s]).astype(bfloat16),
                "wv": np.ascontiguousarray(Wv[:, s]).astype(bfloat16),
                "bqk": np.ascontiguousarray(bqk),
                "bv": np.ascontiguousarray(bv[s][None, :]).astype(bfloat16),
                "wo": np.ascontiguousarray(Wo[s, :]).astype(bfloat16),
            }
        )
    return in_maps


def kernel(
    x,
    bond_influence,
    Wq,
    bq,
    Wk,
    bk,
    Wv,
    bv,
    Wo,
    bo,
    _trace=False,
    _trace_out=None,
):
    x = np.asarray(x, dtype=np.float32)
    bond_influence = np.asarray(bond_influence, dtype=np.float32)
    args = [np.asarray(a, dtype=np.float32) for a in (Wq, bq, Wk, bk, Wv, bv, Wo)]
    bo = np.asarray(bo, dtype=np.float32)

    nc = _get_nc()
    in_maps = _host_prep(x, bond_influence, *args)
    kwargs = {}
    if _trace:
        kwargs = dict(trace=True, tmpdir=_trace_out)
    res = run_bass_kernel_spmd(nc, in_maps, list(range(N_CORES)), **kwargs)

    out = np.zeros((B, L, D), dtype=np.float32)
    for b in range(B):
        acc = res.results[4 * b]["y"].astype(np.float32)
        for g in range(1, HPC):
            acc = acc + res.results[4 * b + g]["y"].astype(np.float32)
        out[b] = acc + bo[None, :]
    if _trace:
        return out, res
    return out


# revision 16
# speedup vs baseline: 1.1362x; 1.0495x over previous
"""BondInfluenceSelfAttention TRN2 kernel (v2).

Full-input contract: kernel(**inputs) takes the complete unsharded inputs and
returns the full [B, L, D] output. Internally shards across 8 NeuronCores:
core c handles batch b = c // 4 and head-group g = c % 4 (4 heads, 256 dk
dims). Each core computes its heads' attention plus the partial output
projection through its 256 rows of Wo; the host sums the 4 partials per batch
and adds bo.

v2 design (vs the v1 in git history):
- bf16 operands for every matmul (proj, scores, PV, out-proj); fp32 PSUM.
- Score matmuls (K=64) packed two-per-array via tile_position row groups;
  PV matmuls (M=64) packed two-per-array via column groups.
- Softmax denominators from 4-way column-packed ones^T @ P matmuls into one
  PSUM bank; normalization deferred to the chunk tail (reciprocal_approx_fast
  on DVE + K=1 broadcast matmuls), killing v1's 53us of DVE RECIPROCAL.
- ACT does only: exp in [128,4096] ops, Q/K/V PSUM->SBUF copies (with bias),
  Y copies. DVE does only: bond-multiply (PSUM->SBUF), recip, normalize.
- QKV projection matmul groups are interleaved into chunk 0's attention steps
  (just-in-time spans) so DVE/ACT never sit idle behind a serial proj phase.
"""

import numpy as np

try:
    import concourse.bass as bass  # noqa: F401
except ImportError:  # pragma: no cover
    import sys

    sys.path.insert(0, "/opt/trn_rl_repo")
    import concourse.bass as bass  # noqa: F401

import concourse.bacc as bacc
import concourse.mybir as mybir
import concourse.tile as tile
from concourse.bass_utils import run_bass_kernel_spmd
from ml_dtypes import bfloat16

F32 = mybir.dt.float32
F32R = mybir.dt.float32r
BF16 = mybir.dt.bfloat16
F16 = mybir.dt.float16

D = 1024  # d_model
L = 2048  # sequence length
B = 2  # batch
HPC = 4  # heads per core
DKG = 256  # dk dims per core (4 heads x 64)
NK = D // 128  # 8 contraction k-tiles for the projections
LT = L // 128  # 16 L k-position tiles
NCH = L // 512  # 4 L_q chunks
NSP = 4  # 512-wide k-position spans (kt/vt production granularity)
N_CORES = 8

_CACHED_NC = None


def _build_nc():
    nc = bacc.Bacc("TRN2", target_bir_lowering=False, debug=False, num_devices=N_CORES)

    xt_d = nc.declare_dram_parameter("xt", [D, L], BF16, isOutput=False)
    bd_d = nc.declare_dram_parameter("bd", [L, L], F16, isOutput=False)
    wq_d = nc.declare_dram_parameter("wq", [D, DKG], BF16, isOutput=False)
    wk_d = nc.declare_dram_parameter("wk", [D, DKG], BF16, isOutput=False)
    wv_d = nc.declare_dram_parameter("wv", [D, DKG], BF16, isOutput=False)
    bqk_d = nc.declare_dram_parameter("bqk", [128, 4], F32, isOutput=False)
    bv_d = nc.declare_dram_parameter("bv", [1, DKG], BF16, isOutput=False)
    wo_d = nc.declare_dram_parameter("wo", [DKG, D], BF16, isOutput=False)
    y_d = nc.declare_dram_parameter("y", [L, D], BF16, isOutput=True)

    Exp = mybir.ActivationFunctionType.Exp
    Identity = mybir.ActivationFunctionType.Identity

    with tile.TileContext(nc) as tc:
        with tc.tile_pool(name="persist", bufs=1) as pp, tc.tile_pool(
            name="work", bufs=1
        ) as wk_pool, tc.tile_pool(name="ps", bufs=1, space="PSUM") as ps:
            # ---- persistent SBUF ----
            xk = [
                pp.tile([128, NK, 512], BF16, tag=f"xk{s}", name=f"xk{s}")
                for s in range(NSP)
            ]
            wq_sb = pp.tile([128, NK, DKG], BF16, tag="wq", name="wq_sb")
            wk_sb = pp.tile([128, NK, DKG], BF16, tag="wk", name="wk_sb")
            wv_sb = pp.tile([128, NK, DKG], BF16, tag="wv", name="wv_sb")
            qt = [pp.tile([128, L], BF16, tag=f"qt{t}", name=f"qt{t}") for t in range(2)]
            kt = [pp.tile([128, L], BF16, tag=f"kt{t}", name=f"kt{t}") for t in range(2)]
            vt = pp.tile([128, LT, DKG], BF16, tag="vt", name="vt")
            st = [
                pp.tile([128, 4, 1024], F32, tag=f"st{g}", name=f"st{g}")
                for g in range(2)
            ]
            pt = [
                pp.tile([128, 4, 1024], BF16, tag=f"pt{g}", name=f"pt{g}")
                for g in range(2)
            ]
            wo_sb = pp.tile([128, 2, D], BF16, tag="wo", name="wo_sb")
            bqk_sb = pp.tile([128, 4], F32, tag="bqk", name="bqk_sb")
            bv_sb = pp.tile([1, DKG], BF16, tag="bv", name="bv_sb")
            ones_r = pp.tile([1, 128], BF16, tag="onesr", name="ones_r")
            onesb = pp.tile([128, 128], BF16, tag="onesb", name="onesb")
            ones_f = pp.tile([128, 128], F32, tag="onesf", name="ones_f")
            # sel[:, t, :]: K=128 selection matrix broadcasting denominator
            # rows {64t, 64t+32} of dsb to output partitions [0:64], [64:128]
            sel = pp.tile([128, 2, 128], F32R, tag="sel", name="sel")
            sel_f = pp.tile([128, 2, 128], F32, tag="self", name="sel_f")

            # ---- input DMA: weights first (gate first proj groups), x spans
            # batched one-per-span and spread across engine queues ----
            nc.sync.dma_start(out=wk_sb, in_=wk_d.ap().rearrange("(k p) n -> p k n", p=128))
            nc.scalar.dma_start(out=wq_sb, in_=wq_d.ap().rearrange("(k p) n -> p k n", p=128))
            nc.gpsimd.dma_start(out=wv_sb, in_=wv_d.ap().rearrange("(k p) n -> p k n", p=128))
            nc.gpsimd.dma_start(out=wo_sb, in_=wo_d.ap().rearrange("(t p) n -> p t n", p=128))
            nc.scalar.dma_start(out=bqk_sb, in_=bqk_d[:, :])
            nc.scalar.dma_start(out=bv_sb, in_=bv_d[:, :])
            xt_ap = xt_d.ap().rearrange("(k p) l -> p k l", p=128)
            span_q = [nc.sync, nc.scalar, nc.gpsimd, nc.sync]
            for s in range(NSP):
                span_q[s].dma_start(out=xk[s], in_=xt_ap[:, :, 512 * s : 512 * (s + 1)])
            nc.vector.memset(ones_f, 1.0)
            nc.vector.tensor_copy(out=onesb, in_=ones_f)
            nc.vector.tensor_copy(out=ones_r, in_=ones_f[0:1, :])
            nc.vector.memset(sel_f, 0.0)
            for t in range(2):
                nc.vector.memset(sel_f[64 * t : 64 * t + 1, t, 0:64], 1.0)
                nc.vector.memset(sel_f[64 * t + 32 : 64 * t + 33, t, 64:128], 1.0)
            nc.vector.tensor_copy(out=sel, in_=sel_f)

            # ---- projection group emitters ----
            def kt_group(t, s):
                pb = ps.tile([128, 512], F32, tag="pj", name="pj")
                for k in range(NK):
                    nc.tensor.matmul(
                        pb[:, :],
                        wk_sb[:, k, 128 * t : 128 * (t + 1)],
                        xk[s][:, k, :],
                        start=(k == 0),
                        stop=(k == NK - 1),
                    )
                nc.scalar.activation(
                    out=kt[t][:, 512 * s : 512 * (s + 1)],
                    in_=pb[:, :],
                    func=Identity,
                    bias=bqk_sb[:, 2 + t : 3 + t],
                )

            def qt_group(t, c):
                pb = ps.tile([128, 512], F32, tag="pj", name="pj")
                for k in range(NK):
                    nc.tensor.matmul(
                        pb[:, :],
                        wq_sb[:, k, 128 * t : 128 * (t + 1)],
                        xk[c][:, k, :],
                        start=(k == 0),
                        stop=(k == NK - 1),
                    )
                nc.scalar.activation(
                    out=qt[t][:, 512 * c : 512 * (c + 1)],
                    in_=pb[:, :],
                    func=Identity,
                    bias=bqk_sb[:, t : t + 1],
                )

            def vt_group(ii):  # i-tiles 2*ii, 2*ii+1
                pb = ps.tile([128, 512], F32, tag="pj", name="pj")
                for j in range(2):
                    i = 2 * ii + j
                    s, lo = i // 4, (i % 4) * 128
                    for k in range(NK):
                        nc.tensor.matmul(
                            pb[:, 256 * j : 256 * (j + 1)],
                            xk[s][:, k, lo : lo + 128],
                            wv_sb[:, k, :],
                            start=(k == 0),
                            stop=False,
                        )
                    nc.tensor.matmul(
                        pb[:, 256 * j : 256 * (j + 1)],
                        ones_r,
                        bv_sb,
                        start=False,
                        stop=True,
                    )
                nc.scalar.activation(
                    out=vt[:, 2 * ii : 2 * ii + 2, :],
                    in_=pb.rearrange("p (j n) -> p j n", j=2),
                    func=Identity,
                )

            # just-in-time schedule: chunk-0 steps produce the remaining spans
            slots = {
                (0, 4): [lambda: kt_group(0, 2)],
                (0, 5): [lambda: kt_group(1, 2)],
                (0, 6): [lambda: vt_group(4)],
                (0, 7): [lambda: vt_group(5)],
                (0, 8): [lambda: kt_group(0, 3)],
                (0, 9): [lambda: kt_group(1, 3)],
                (0, 10): [lambda: vt_group(6)],
                (0, 12): [lambda: vt_group(7)],
                (0, 13): [lambda: qt_group(0, 1)],
                (0, 14): [lambda: qt_group(1, 1)],
                (1, 2): [lambda: qt_group(0, 2)],
                (1, 6): [lambda: qt_group(1, 2)],
                (2, 2): [lambda: qt_group(0, 3)],
                (2, 6): [lambda: qt_group(1, 3)],
            }

            # bootstrap (overlapped with the x-span DMAs): spans 0-1 + qt c0
            kt_group(0, 0)
            kt_group(1, 0)
            qt_group(0, 0)
            qt_group(1, 0)
            vt_group(0)
            vt_group(1)
            kt_group(0, 1)
            kt_group(1, 1)
            vt_group(2)
            vt_group(3)

            # ---- bond DMA ring ----
            steps = [(c, i) for c in range(NCH) for i in range(LT)]
            bts = {}

            def bond_dma(n):
                if n >= len(steps):
                    return
                c, i = steps[n]
                bt = wk_pool.tile([128, 512], F16, tag="bt", bufs=6, name="bt")
                nc.sync.dma_start(
                    out=bt,
                    in_=bd_d[128 * i : 128 * (i + 1), 512 * c : 512 * (c + 1)],
                )
                bts[n] = bt

            for n in range(3):
                bond_dma(n)

            # ---- attention ----
            def pv_step(c, j, oacc, dn):
                first, last = (j == 0), (j == LT - 1)
                g = (j // 2) % 2
                for t in range(2):
                    idx = (j % 2) * 2 + t
                    for half in range(2):
                        h = 2 * t + half
                        nc.tensor.matmul(
                            oacc[t][64 * half : 64 * (half + 1), :],
                            vt[:, j, 64 * h : 64 * (h + 1)],
                            pt[g][:, idx, 512 * half : 512 * (half + 1)],
                            start=first,
                            stop=last,
                            tile_position=(0, 64 * half),
                        )
                for h in range(HPC):
                    t, half = h // 2, h % 2
                    idx = (j % 2) * 2 + t
                    nc.tensor.matmul(
                        dn[32 * h : 32 * h + 1, :],
                        onesb[:, 0:1],
                        pt[g][:, idx, 512 * half : 512 * (half + 1)],
                        start=first,
                        stop=last,
                        tile_position=(0, 32 * h),
                    )

            for c in range(NCH):
                oacc = [
                    ps.tile([128, 512], F32, tag=f"o{t}", name=f"oacc{t}")
                    for t in range(2)
                ]
                dn = ps.tile([128, 512], F32, tag="dn", name="dn")
                for i in range(LT):
                    n = c * LT + i
                    for fn in slots.get((c, i), ()):
                        fn()
                    bond_dma(n + 3)
                    bt = bts.pop(n)
                    bt_b = bass.AP(
                        tensor=bt.tensor,
                        offset=bt.offset,
                        ap=[bt.ap[0], [0, 2]] + list(bt.ap[1:]),
                    )
                    g = (i // 2) % 2
                    for t in range(2):
                        sp = ps.tile([128, 2, 512], F32, tag="s", bufs=2, name="sp")
                        nc.tensor.matmul(
                            sp[:, 0, :],
                            kt[t][0:64, 128 * i : 128 * (i + 1)],
                            qt[t][0:64, 512 * c : 512 * (c + 1)],
                            start=True,
                            stop=True,
                        )
                        nc.tensor.matmul(
                            sp[:, 1, :],
                            kt[t][64:128, 128 * i : 128 * (i + 1)],
                            qt[t][64:128, 512 * c : 512 * (c + 1)],
                            start=True,
                            stop=True,
                            tile_position=(64, 0),
                        )
                        idx = (i % 2) * 2 + t
                        out_view = st[g][:, idx, :].rearrange("p (h q) -> p h q", h=2)
                        nc.vector.tensor_mul(out=out_view, in0=sp, in1=bt_b)
                    if i % 2 == 1:
                        with nc.allow_low_precision(reason="bf16 probs"):
                            nc.scalar.activation(out=pt[g], in_=st[g], func=Exp)
                    if i >= 2:
                        pv_step(c, i - 2, oacc, dn)
                pv_step(c, LT - 2, oacc, dn)
                pv_step(c, LT - 1, oacc, dn)

                # ---- chunk tail: denominators, normalize, out-proj ----
                dsb = wk_pool.tile([128, 512], F32R, tag="dsb", bufs=2, name="dsb")
                nc.scalar.activation(out=dsb, in_=dn, func=Identity)
                bcb = [
                    ps.tile([128, 512], F32, tag=("dn" if t == 0 else "pj"), name="bcb")
                    for t in range(2)
                ]
                for t in range(2):
                    nc.tensor.matmul(
                        bcb[t][:, :],
                        sel[:, t, :],
                        dsb[:, :],
                        start=True,
                        stop=True,
                    )
                rb = [
                    wk_pool.tile([128, 512], F32, tag="rb", bufs=2, name="rb")
                    for _ in range(2)
                ]
                on = wk_pool.tile([128, 2, 512], BF16, tag="on", bufs=2, name="on")
                with nc.allow_low_precision(reason="bf16 normalized O"):
                    for t in range(2):
                        nc.vector.reciprocal_approx_fast(out=rb[t], in_=bcb[t])
                        nc.vector.tensor_mul(out=on[:, t, :], in0=oacc[t], in1=rb[t])
                for jl in range(4):
                    j = 4 * c + jl
                    for dh in range(2):
                        yp = ps.tile(
                            [128, 512],
                            F32,
                            tag=("pj" if (2 * jl + dh) % 2 else "dn"),
                            name="yp",
                        )
                        for t in range(2):
                            nc.tensor.matmul(
                                yp[:, :],
                                on[:, t, 128 * jl : 128 * (jl + 1)],
                                wo_sb[:, t, 512 * dh : 512 * (dh + 1)],
                                start=(t == 0),
                                stop=(t == 1),
                            )
                        ys = wk_pool.tile([128, 512], BF16, tag="ys", bufs=4, name="ys")
                        with nc.allow_low_precision(reason="bf16 partial Y"):
                            nc.scalar.activation(out=ys, in_=yp, func=Identity)
                        nc.gpsimd.dma_start(
                            out=y_d[128 * j : 128 * (j + 1), 512 * dh : 512 * (dh + 1)],
                            in_=ys,
                        )

    nc.compile()
    return nc


def _get_nc():
    global _CACHED_NC
    if _CACHED_NC is None:
        _CACHED_NC = _build_nc()
    return _CACHED_NC


def _host_prep(x, bond_influence, Wq, bq, Wk, bk, Wv, bv, Wo):
    in_maps = []
    for core in range(N_CORES):
        b, g = core // HPC, core % HPC
        s = slice(g * DKG, (g + 1) * DKG)
        bq_g = (bq[s] / 8.0).astype(np.float32)
        bk_g = bk[s].astype(np.float32)
        bqk = np.stack(
            [bq_g[0:128], bq_g[128:256], bk_g[0:128], bk_g[128:256]], axis=1
        )
        in_maps.append(
            {
                "xt": np.ascontiguousarray(x[b].T).astype(bfloat16),
                "bd": np.ascontiguousarray(bond_influence[b].T.astype(np.float16)),
                "wq": np.ascontiguousarray(Wq[:, s] / 8.0).astype(bfloat16),
                "wk": np.ascontiguousarray(Wk[:, s]).astype(bfloat16),
                "wv": np.ascontiguousarray(Wv[:, s]).astype(bfloat16),
                "bqk": np.ascontiguousarray(bqk),
                "bv": np.ascontiguousarray(bv[s][None, :]).astype(bfloat16),
                "wo": np.ascontiguousarray(Wo[s, :]).astype(bfloat16),
            }
        )
    return in_maps


def kernel(
    x,
    bond_influence,
    Wq,
    bq,
    Wk,
    bk,
    Wv,
    bv,
    Wo,
    bo,
    _trace=False,
    _trace_out=None,
):
    x = np.asarray(x, dtype=np.float32)
    bond_influence = np.asarray(bond_influence, dtype=np.float32)
    args = [np.asarray(a, dtype=np.float32) for a in (Wq, bq, Wk, bk, Wv, bv, Wo)]
    bo = np.asarray(bo, dtype=np.float32)

    nc = _get_nc()
    in_maps = _host_prep(x, bond_influence, *args)
    kwargs = {}
    if _trace:
        kwargs = dict(trace=True, tmpdir=_trace_out)
    res = run_bass_kernel_spmd(nc, in_maps, list(range(N_CORES)), **kwargs)

    out = np.zeros((B, L, D), dtype=np.float32)
    for b in range(B):
        acc = res.results[4 * b]["y"].astype(np.float32)
        for g in range(1, HPC):
            acc = acc + res.results[4 * b + g]["y"].astype(np.float32)
        out[b] = acc + bo[None, :]
    if _trace:
        return out, res
    return out


# revision 20
# speedup vs baseline: 1.1615x; 1.0223x over previous
"""BondInfluenceSelfAttention TRN2 kernel (v2).

Full-input contract: kernel(**inputs) takes the complete unsharded inputs and
returns the full [B, L, D] output. Internally shards across 8 NeuronCores:
core c handles batch b = c // 4 and head-group g = c % 4 (4 heads, 256 dk
dims). Each core computes its heads' attention plus the partial output
projection through its 256 rows of Wo; the host sums the 4 partials per batch
and adds bo.

v2 design (vs the v1 in git history):
- bf16 operands for every matmul (proj, scores, PV, out-proj); fp32 PSUM.
- Score matmuls (K=64) packed two-per-array via tile_position row groups;
  PV matmuls (M=64) packed two-per-array via column groups.
- Softmax denominators from 4-way column-packed ones^T @ P matmuls into one
  PSUM bank; normalization deferred to the chunk tail (reciprocal_approx_fast
  on DVE + K=1 broadcast matmuls), killing v1's 53us of DVE RECIPROCAL.
- ACT does only: exp in [128,4096] ops, Q/K/V PSUM->SBUF copies (with bias),
  Y copies. DVE does only: bond-multiply (PSUM->SBUF), recip, normalize.
- QKV projection matmul groups are interleaved into chunk 0's attention steps
  (just-in-time spans) so DVE/ACT never sit idle behind a serial proj phase.
"""

import numpy as np

try:
    import concourse.bass as bass  # noqa: F401
except ImportError:  # pragma: no cover
    import sys

    sys.path.insert(0, "/opt/trn_rl_repo")
    import concourse.bass as bass  # noqa: F401

import concourse.bacc as bacc
import concourse.mybir as mybir
import concourse.tile as tile
from concourse.bass_utils import run_bass_kernel_spmd
from ml_dtypes import bfloat16

F32 = mybir.dt.float32
F32R = mybir.dt.float32r
BF16 = mybir.dt.bfloat16
F16 = mybir.dt.float16

D = 1024  # d_model
L = 2048  # sequence length
B = 2  # batch
HPC = 4  # heads per core
DKG = 256  # dk dims per core (4 heads x 64)
NK = D // 128  # 8 contraction k-tiles for the projections
LT = L // 128  # 16 L k-position tiles
NCH = L // 512  # 4 L_q chunks
NSP = 4  # 512-wide k-position spans (kt/vt production granularity)
N_CORES = 8

_CACHED_NC = None


def _build_nc():
    nc = bacc.Bacc("TRN2", target_bir_lowering=False, debug=False, num_devices=N_CORES)

    xt_d = nc.declare_dram_parameter("xt", [D, L], BF16, isOutput=False)
    bd_d = nc.declare_dram_parameter("bd", [L, L], F16, isOutput=False)
    wq_d = nc.declare_dram_parameter("wq", [D, DKG], BF16, isOutput=False)
    wk_d = nc.declare_dram_parameter("wk", [D, DKG], BF16, isOutput=False)
    wv_d = nc.declare_dram_parameter("wv", [D, DKG], BF16, isOutput=False)
    bqk_d = nc.declare_dram_parameter("bqk", [128, 4], F32, isOutput=False)
    bv_d = nc.declare_dram_parameter("bv", [1, DKG], BF16, isOutput=False)
    wo_d = nc.declare_dram_parameter("wo", [DKG, D], BF16, isOutput=False)
    y_d = nc.declare_dram_parameter("y", [L, D], BF16, isOutput=True)

    Exp = mybir.ActivationFunctionType.Exp
    Identity = mybir.ActivationFunctionType.Identity

    with tile.TileContext(nc) as tc:
        with tc.tile_pool(name="persist", bufs=1) as pp, tc.tile_pool(
            name="work", bufs=1
        ) as wk_pool, tc.tile_pool(name="ps", bufs=1, space="PSUM") as ps:
            # ---- persistent SBUF ----
            xk = [
                pp.tile([128, NK, 512], BF16, tag=f"xk{s}", name=f"xk{s}")
                for s in range(NSP)
            ]
            wq_sb = pp.tile([128, NK, DKG], BF16, tag="wq", name="wq_sb")
            wk_sb = pp.tile([128, NK, DKG], BF16, tag="wk", name="wk_sb")
            wv_sb = pp.tile([128, NK, DKG], BF16, tag="wv", name="wv_sb")
            qt = [pp.tile([128, L], BF16, tag=f"qt{t}", name=f"qt{t}") for t in range(2)]
            kt = [pp.tile([128, L], BF16, tag=f"kt{t}", name=f"kt{t}") for t in range(2)]
            vt = pp.tile([128, LT, DKG], BF16, tag="vt", name="vt")
            st = [
                pp.tile([128, 4, 1024], F32, tag=f"st{g}", name=f"st{g}")
                for g in range(2)
            ]
            pt = [
                pp.tile([128, 4, 1024], BF16, tag=f"pt{g}", name=f"pt{g}")
                for g in range(2)
            ]
            wo_sb = pp.tile([128, 2, D], BF16, tag="wo", name="wo_sb")
            bqk_sb = pp.tile([128, 4], F32, tag="bqk", name="bqk_sb")
            bv_sb = pp.tile([1, DKG], BF16, tag="bv", name="bv_sb")
            ones_r = pp.tile([1, 128], BF16, tag="onesr", name="ones_r")
            onesb = pp.tile([128, 128], BF16, tag="onesb", name="onesb")
            ones_f = pp.tile([128, 128], F32, tag="onesf", name="ones_f")
            # sel[:, t, :]: K=128 selection matrix broadcasting denominator
            # rows {64t, 64t+32} of dsb to output partitions [0:64], [64:128]
            sel = pp.tile([128, 2, 128], F32R, tag="sel", name="sel")
            sel_f = pp.tile([128, 2, 128], F32, tag="self", name="sel_f")

            # ---- input DMA: weights first (gate first proj groups), x spans
            # batched one-per-span and spread across engine queues ----
            # span 0 split per-k across all three queues so the first proj
            # group can start ~6us in; spans 1-3 as half-DMAs on scalar/gpsimd
            # so the sync queue is free for bond tiles.
            nc.sync.dma_start(out=wk_sb, in_=wk_d.ap().rearrange("(k p) n -> p k n", p=128))
            nc.scalar.dma_start(out=wq_sb, in_=wq_d.ap().rearrange("(k p) n -> p k n", p=128))
            nc.gpsimd.dma_start(out=wv_sb, in_=wv_d.ap().rearrange("(k p) n -> p k n", p=128))
            nc.scalar.dma_start(out=bqk_sb, in_=bqk_d[:, :])
            nc.scalar.dma_start(out=bv_sb, in_=bv_d[:, :])
            qs = [nc.sync, nc.scalar, nc.gpsimd]
            for k in range(NK):
                qs[k % 3].dma_start(
                    out=xk[0][:, k, :], in_=xt_d[128 * k : 128 * (k + 1), 0:512]
                )
            nc.gpsimd.dma_start(out=wo_sb, in_=wo_d.ap().rearrange("(t p) n -> p t n", p=128))
            xt_ap = xt_d.ap().rearrange("(k p) l -> p k l", p=128)
            for s in range(1, NSP):
                for h, q in ((0, nc.scalar), (1, nc.gpsimd)):
                    q.dma_start(
                        out=xk[s][:, 4 * h : 4 * (h + 1), :],
                        in_=xt_ap[:, 4 * h : 4 * (h + 1), 512 * s : 512 * (s + 1)],
                    )
            nc.vector.memset(ones_f, 1.0)
            nc.vector.tensor_copy(out=onesb, in_=ones_f)
            nc.vector.tensor_copy(out=ones_r, in_=ones_f[0:1, :])
            nc.vector.memset(sel_f, 0.0)
            for t in range(2):
                nc.vector.memset(sel_f[64 * t : 64 * t + 1, t, 0:64], 1.0)
                nc.vector.memset(sel_f[64 * t + 32 : 64 * t + 33, t, 64:128], 1.0)
            nc.vector.tensor_copy(out=sel, in_=sel_f)

            # ---- projection group emitters ----
            def kt_group(t, s):
                pb = ps.tile([128, 512], F32, tag="pj", name="pj")
                for k in range(NK):
                    nc.tensor.matmul(
                        pb[:, :],
                        wk_sb[:, k, 128 * t : 128 * (t + 1)],
                        xk[s][:, k, :],
                        start=(k == 0),
                        stop=(k == NK - 1),
                    )
                nc.scalar.activation(
                    out=kt[t][:, 512 * s : 512 * (s + 1)],
                    in_=pb[:, :],
                    func=Identity,
                    bias=bqk_sb[:, 2 + t : 3 + t],
                )

            def qt_group(t, c):
                pb = ps.tile([128, 512], F32, tag="pj", name="pj")
                for k in range(NK):
                    nc.tensor.matmul(
                        pb[:, :],
                        wq_sb[:, k, 128 * t : 128 * (t + 1)],
                        xk[c][:, k, :],
                        start=(k == 0),
                        stop=(k == NK - 1),
                    )
                nc.scalar.activation(
                    out=qt[t][:, 512 * c : 512 * (c + 1)],
                    in_=pb[:, :],
                    func=Identity,
                    bias=bqk_sb[:, t : t + 1],
                )

            def vt_group(ii):  # i-tiles 2*ii, 2*ii+1
                pb = ps.tile([128, 512], F32, tag="pj", name="pj")
                for j in range(2):
                    i = 2 * ii + j
                    s, lo = i // 4, (i % 4) * 128
                    for k in range(NK):
                        nc.tensor.matmul(
                            pb[:, 256 * j : 256 * (j + 1)],
                            xk[s][:, k, lo : lo + 128],
                            wv_sb[:, k, :],
                            start=(k == 0),
                            stop=False,
                        )
                    nc.tensor.matmul(
                        pb[:, 256 * j : 256 * (j + 1)],
                        ones_r,
                        bv_sb,
                        start=False,
                        stop=True,
                    )
                nc.scalar.activation(
                    out=vt[:, 2 * ii : 2 * ii + 2, :],
                    in_=pb.rearrange("p (j n) -> p j n", j=2),
                    func=Identity,
                )

            # fine-grained qt emitter: 2 matmuls per slot, psum bank held
            # across the slots of one group to avoid 8-MM pacer stalls
            qt_pb = {}

            def qt_part(t, c, k0, k1):
                key = (t, c)
                if key not in qt_pb:
                    qt_pb[key] = ps.tile([128, 512], F32, tag="pj", name="pj")
                pb = qt_pb[key]
                for k in range(k0, k1):
                    nc.tensor.matmul(
                        pb[:, :],
                        wq_sb[:, k, 128 * t : 128 * (t + 1)],
                        xk[c][:, k, :],
                        start=(k == 0),
                        stop=(k == NK - 1),
                    )
                if k1 == NK:
                    nc.scalar.activation(
                        out=qt[t][:, 512 * c : 512 * (c + 1)],
                        in_=pb[:, :],
                        func=Identity,
                        bias=bqk_sb[:, t : t + 1],
                    )
                    del qt_pb[key]

            # just-in-time schedule: chunk-0 steps produce the remaining spans
            slots = {
                (0, 0): [lambda: vt_group(0)],
                (0, 1): [lambda: vt_group(1)],
                (0, 2): [lambda: kt_group(0, 1)],
                (0, 3): [lambda: kt_group(1, 1)],
                (0, 4): [lambda: vt_group(2)],
                (0, 5): [lambda: vt_group(3)],
                (0, 6): [lambda: kt_group(0, 2)],
                (0, 7): [lambda: kt_group(1, 2)],
                (0, 8): [lambda: vt_group(4)],
                (0, 9): [lambda: vt_group(5)],
                (0, 10): [lambda: kt_group(0, 3)],
                (0, 11): [lambda: kt_group(1, 3)],
                (0, 12): [lambda: vt_group(6)],
                (0, 13): [lambda: vt_group(7)],
                (0, 14): [lambda: qt_group(0, 1)],
                (0, 15): [lambda: qt_group(1, 1)],
            }
            for c in (1, 2):
                cn = c + 1
                for p in range(4):
                    slots[(c, 2 + p)] = [
                        lambda t=0, cn=cn, p=p: qt_part(t, cn, 2 * p, 2 * p + 2)
                    ]
                    slots[(c, 6 + p)] = [
                        lambda t=1, cn=cn, p=p: qt_part(t, cn, 2 * p, 2 * p + 2)
                    ]

            # bootstrap (overlapped with the span-0 DMA): kt span 0 + qt c0
            kt_group(0, 0)
            kt_group(1, 0)
            qt_group(0, 0)
            qt_group(1, 0)

            # ---- bond DMA ring ----
            steps = [(c, i) for c in range(NCH) for i in range(LT)]
            bts = {}

            def bond_dma(n):
                if n >= len(steps):
                    return
                c, i = steps[n]
                bt = wk_pool.tile([128, 512], F16, tag="bt", bufs=6, name="bt")
                nc.sync.dma_start(
                    out=bt,
                    in_=bd_d[128 * i : 128 * (i + 1), 512 * c : 512 * (c + 1)],
                )
                bts[n] = bt

            for n in range(4):
                bond_dma(n)

            # ---- attention ----
            def pv_step(c, j, oacc, dn):
                first, last = (j == 0), (j == LT - 1)
                g = (j // 2) % 2
                for t in range(2):
                    idx = (j % 2) * 2 + t
                    for half in range(2):
                        h = 2 * t + half
                        nc.tensor.matmul(
                            oacc[t][64 * half : 64 * (half + 1), :],
                            vt[:, j, 64 * h : 64 * (h + 1)],
                            pt[g][:, idx, 512 * half : 512 * (half + 1)],
                            start=first,
                            stop=last,
                            tile_position=(0, 64 * half),
                        )
                for h in range(HPC):
                    t, half = h // 2, h % 2
                    idx = (j % 2) * 2 + t
                    nc.tensor.matmul(
                        dn[32 * h : 32 * h + 1, :],
                        onesb[:, 0:1],
                        pt[g][:, idx, 512 * half : 512 * (half + 1)],
                        start=first,
                        stop=last,
                        tile_position=(0, 32 * h),
                    )

            for c in range(NCH):
                oacc = [
                    ps.tile([128, 512], F32, tag=f"o{t}", name=f"oacc{t}")
                    for t in range(2)
                ]
                dn = ps.tile([128, 512], F32, tag="dn", name="dn")
                for i in range(LT):
                    n = c * LT + i
                    for fn in slots.get((c, i), ()):
                        fn()
                    bond_dma(n + 4)
                    bt = bts.pop(n)
                    bt_b = bass.AP(
                        tensor=bt.tensor,
                        offset=bt.offset,
                        ap=[bt.ap[0], [0, 2]] + list(bt.ap[1:]),
                    )
                    g = (i // 2) % 2
                    for t in range(2):
                        sp = ps.tile([128, 2, 512], F32, tag="s", bufs=2, name="sp")
                        nc.tensor.matmul(
                            sp[:, 0, :],
                            kt[t][0:64, 128 * i : 128 * (i + 1)],
                            qt[t][0:64, 512 * c : 512 * (c + 1)],
                            start=True,
                            stop=True,
                        )
                        nc.tensor.matmul(
                            sp[:, 1, :],
                            kt[t][64:128, 128 * i : 128 * (i + 1)],
                            qt[t][64:128, 512 * c : 512 * (c + 1)],
                            start=True,
                            stop=True,
                            tile_position=(64, 0),
                        )
                        idx = (i % 2) * 2 + t
                        out_view = st[g][:, idx, :].rearrange("p (h q) -> p h q", h=2)
                        nc.vector.tensor_mul(out=out_view, in0=sp, in1=bt_b)
                    if i % 2 == 1:
                        with nc.allow_low_precision(reason="bf16 probs"):
                            nc.scalar.activation(out=pt[g], in_=st[g], func=Exp)
                    if i >= 2:
                        pv_step(c, i - 2, oacc, dn)
                pv_step(c, LT - 2, oacc, dn)
                pv_step(c, LT - 1, oacc, dn)

                # ---- chunk tail: denominators, normalize, out-proj ----
                dsb = wk_pool.tile([128, 512], F32R, tag="dsb", bufs=2, name="dsb")
                nc.scalar.activation(out=dsb, in_=dn, func=Identity)
                bcb = [
                    ps.tile([128, 512], F32, tag=("dn" if t == 0 else "pj"), name="bcb")
                    for t in range(2)
                ]
                for t in range(2):
                    nc.tensor.matmul(
                        bcb[t][:, :],
                        sel[:, t, :],
                        dsb[:, :],
                        start=True,
                        stop=True,
                    )
                rb = [
                    wk_pool.tile([128, 512], F32, tag="rb", bufs=2, name="rb")
                    for _ in range(2)
                ]
                on = wk_pool.tile([128, 2, 512], BF16, tag="on", bufs=2, name="on")
                with nc.allow_low_precision(reason="bf16 normalized O"):
                    for t in range(2):
                        nc.vector.reciprocal_approx_fast(out=rb[t], in_=bcb[t])
                        nc.vector.tensor_mul(out=on[:, t, :], in0=oacc[t], in1=rb[t])
                for jl in range(4):
                    j = 4 * c + jl
                    for dh in range(2):
                        yp = ps.tile(
                            [128, 512],
                            F32,
                            tag=("pj" if (2 * jl + dh) % 2 else "dn"),
                            name="yp",
                        )
                        for t in range(2):
                            nc.tensor.matmul(
                                yp[:, :],
                                on[:, t, 128 * jl : 128 * (jl + 1)],
                                wo_sb[:, t, 512 * dh : 512 * (dh + 1)],
                                start=(t == 0),
                                stop=(t == 1),
                            )
                        ys = wk_pool.tile([128, 512], BF16, tag="ys", bufs=4, name="ys")
                        with nc.allow_low_precision(reason="bf16 partial Y"):
                            nc.scalar.activation(out=ys, in_=yp, func=Identity)
                        nc.gpsimd.dma_start(
                            out=y_d[128 * j : 128 * (j + 1), 512 * dh : 512 * (dh + 1)],
                            in_=ys,
                        )

    nc.compile()
    return nc


def _get_nc():
    global _CACHED_NC
    if _CACHED_NC is None:
        _CACHED_NC = _build_nc()
    return _CACHED_NC


def _host_prep(x, bond_influence, Wq, bq, Wk, bk, Wv, bv, Wo):
    in_maps = []
    for core in range(N_CORES):
        b, g = core // HPC, core % HPC
        s = slice(g * DKG, (g + 1) * DKG)
        bq_g = (bq[s] / 8.0).astype(np.float32)
        bk_g = bk[s].astype(np.float32)
        bqk = np.stack(
            [bq_g[0:128], bq_g[128:256], bk_g[0:128], bk_g[128:256]], axis=1
        )
        in_maps.append(
            {
                "xt": np.ascontiguousarray(x[b].T).astype(bfloat16),
                "bd": np.ascontiguousarray(bond_influence[b].T.astype(np.float16)),
                "wq": np.ascontiguousarray(Wq[:, s] / 8.0).astype(bfloat16),
                "wk": np.ascontiguousarray(Wk[:, s]).astype(bfloat16),
                "wv": np.ascontiguousarray(Wv[:, s]).astype(bfloat16),
                "bqk": np.ascontiguousarray(bqk),
                "bv": np.ascontiguousarray(bv[s][None, :]).astype(bfloat16),
                "wo": np.ascontiguousarray(Wo[s, :]).astype(bfloat16),
            }
        )
    return in_maps


def kernel(
    x,
    bond_influence,
    Wq,
    bq,
    Wk,
    bk,
    Wv,
    bv,
    Wo,
    bo,
    _trace=False,
    _trace_out=None,
):
    x = np.asarray(x, dtype=np.float32)
    bond_influence = np.asarray(bond_influence, dtype=np.float32)
    args = [np.asarray(a, dtype=np.float32) for a in (Wq, bq, Wk, bk, Wv, bv, Wo)]
    bo = np.asarray(bo, dtype=np.float32)

    nc = _get_nc()
    in_maps = _host_prep(x, bond_influence, *args)
    kwargs = {}
    if _trace:
        kwargs = dict(trace=True, tmpdir=_trace_out)
    res = run_bass_kernel_spmd(nc, in_maps, list(range(N_CORES)), **kwargs)

    out = np.zeros((B, L, D), dtype=np.float32)
    for b in range(B):
        acc = res.results[4 * b]["y"].astype(np.float32)
        for g in range(1, HPC):
            acc = acc + res.results[4 * b + g]["y"].astype(np.float32)
        out[b] = acc + bo[None, :]
    if _trace:
        return out, res
    return out
